# revision 11
# baseline (speedup 1.0000x reference)
"""MoE block (8 experts, top-2, + shared expert) on 8 trn2 NeuronCores.

Strategy (expert-parallel, host dispatch):
  - Host computes gate logits/softmax/top-2 (0.03% of total FLOPs) and
    dispatches tokens: core c receives the tokens routed to expert c
    (padded to the max per-expert count) plus a 1/8 slice of all tokens
    for the shared expert.
  - Each core runs two FFN passes in one Bass program: shared FFN on its
    512-token slice, then expert-c FFN on its routed tokens. Matmuls are
    bf16 (weights + activations) with fp32 PSUM accumulation; feature-major
    ([D, tokens]) layout avoids all on-device transposes.
  - Host combines: routed outputs scaled by renormalized top-2 weights and
    scatter-added, shared outputs added per-slice, biases b2/bs2 added on
    host (they enter linearly).

DMA layout: all weight/x tensors are host-packed so every transfer has
>=8KB contiguous per partition (1KB strided lines measured ~11 GB/s per
queue; 8KB lines approach the 341 GB/s 1MB-transfer rate):
  w1p [128, 32768]: cols (g, d, j) so one per-g transfer [128, 4096]
      yields lhsT slices [:, d*512+fi*128 :+128] (partition = contraction).
  w2p [128, 32768]: cols (fb, ff, j) so one per-fb transfer [128, 8192]
      yields lhsT slices [:, ff*1024+d*128 :+128].
  xp  [128, 8*NT]: per chunk cols (d, t) so one transfer per chunk.
Each dma_start instruction costs ~600ns serialized on its HWDGE ring, so
the startup-critical pair (x chunk 0 + shared-w1 g0) is issued as just two
transfers, one on each of the two HWDGE rings (sync + scalar) so they
issue in parallel; biases and y-output stores also go to the scalar ring
to keep the sync ring free for bulk weight streaming.
"""

import numpy as np
import ml_dtypes

import concourse.bass as bass
import concourse.bacc as bacc
from concourse import mybir
from concourse.tile import TileContext
from concourse.bass_utils import run_bass_kernel_spmd

D = 1024
FF = 4096
E = 8
TOPK = 2
B, L = 4, 1024
T = B * L
NCORES = 8
SHARED = T // NCORES  # shared-expert tokens per core
P = 128
DT = D // P    # 8 k-tiles for D
FT = FF // P   # 32 tiles for FF
FG = 8         # w1 f-groups (one transfer each)
FGW = FF // FG # 512 w1 cols per f-group
FB = 4         # w2 f-blocks (one transfer each)
FBW = FT // FB # 8 f-tiles per w2 block

_BF16 = mybir.dt.bfloat16
_F32 = mybir.dt.float32

_program_cache: dict[tuple, tuple] = {}

# test harness hooks: extra kwargs for run_bass_kernel_spmd (e.g. trace=True)
# and the last BassKernelResults for profiling. Unused in normal grading runs.
TRACE_KWARGS: dict = {}
last_results = None

WARMUP_MM = 28


def _chunk_plan(C: int) -> list[int]:
    """Split C routed columns into <=512-wide chunks, sizes multiple of 8."""
    n = -(-C // 512)
    base = -(-C // n)
    base = -(-base // 8) * 8
    sizes = []
    left = C
    for _ in range(n - 1):
        sizes.append(base)
        left -= base
    sizes.append(left)
    assert all(0 < s <= 512 for s in sizes) and sum(sizes) == C
    return sizes


def _pack_w1(W: np.ndarray) -> np.ndarray:
    """[D, FF] -> [128, FG*DT*FGW] with cols (g, d, j)."""
    return np.ascontiguousarray(
        W.reshape(DT, P, FG, FGW).transpose(1, 2, 0, 3).reshape(P, FG * DT * FGW)
    )


def _pack_w2(W: np.ndarray) -> np.ndarray:
    """[FF, D] -> [128, FB*FBW*D] with cols (fb, ff, j)."""
    return np.ascontiguousarray(
        W.reshape(FB, FBW, P, D).transpose(2, 0, 1, 3).reshape(P, FB * FBW * D)
    )


def _pack_x(xcols: np.ndarray, chunks: list[tuple[int, int]]) -> np.ndarray:
    """[NT, D] bf16 -> [128, DT*NT] with per-chunk col blocks (d, t)."""
    blocks = []
    for off, N in chunks:
        blk = xcols[off:off + N].reshape(N, DT, P).transpose(2, 1, 0)
        blocks.append(blk.reshape(P, DT * N))
    return np.ascontiguousarray(np.concatenate(blocks, axis=1))


def _build_program(C: int):
    """One SPMD Bass program: shared FFN (512 cols) + expert FFN (C cols)."""
    NT = SHARED + C
    nc = bacc.Bacc()

    xp = nc.dram_tensor("xp", [P, DT * NT], _BF16, kind="ExternalInput")
    w1e = nc.dram_tensor("w1e", [P, FG * DT * FGW], _BF16, kind="ExternalInput")
    w2e = nc.dram_tensor("w2e", [P, FB * FBW * D], _BF16, kind="ExternalInput")
    ws1 = nc.dram_tensor("ws1", [P, FG * DT * FGW], _BF16, kind="ExternalInput")
    ws2 = nc.dram_tensor("ws2", [P, FB * FBW * D], _BF16, kind="ExternalInput")
    b1r = nc.dram_tensor("b1r", [P, FT], _F32, kind="ExternalInput")
    bs1r = nc.dram_tensor("bs1r", [P, FT], _F32, kind="ExternalInput")
    yt = nc.dram_tensor("yt", [D, NT], _F32, kind="ExternalOutput")

    # chunks: (weights_key, col_offset, width); shared phase first so the
    # expert weights can stream in (reusing the same SBUF slots) while the
    # shared phase computes.
    chunks = [("s", 0, SHARED)]
    off = SHARED
    for w in _chunk_plan(C):
        chunks.append(("e", off, w))
        off += w
    # packed-x col offset of each chunk: DT * token-offset
    xoffs = {}
    o = 0
    for _, coff, N in chunks:
        xoffs[coff] = o
        o += DT * N

    with TileContext(nc) as tc:
        with (
            tc.tile_pool(name="wpool", bufs=1) as wpool,
            tc.tile_pool(name="xpool", bufs=1) as xpool,
            tc.tile_pool(name="hpool", bufs=34) as hpool,
            tc.tile_pool(name="ypool", bufs=8) as ypool,
            tc.tile_pool(name="bpool", bufs=1) as bpool,
            tc.tile_pool(name="psum", bufs=4, space="PSUM") as psum,
        ):
            def load_w1(src1, pfx, groups, eng=None, halves=False):
                t1 = {}
                for g in groups:
                    t = wpool.tile([P, DT * FGW], _BF16, tag=f"w1_{g}",
                                   name=f"{pfx}w1_{g}")
                    src = src1[:, g * DT * FGW:(g + 1) * DT * FGW]
                    e = eng or (nc.sync if g % 2 == 0 else nc.scalar)
                    if halves:
                        h = DT * FGW // 2
                        e.dma_start(t[:, :h], src[:, :h])
                        e.dma_start(t[:, h:], src[:, h:])
                    else:
                        e.dma_start(t, src)
                    t1[g] = t
                return t1

            def load_w2(src2, pfx):
                t2 = []
                for fb in range(FB):
                    t = wpool.tile([P, FBW * D], _BF16, tag=f"w2_{fb}",
                                   name=f"{pfx}w2_{fb}")
                    e = nc.sync if fb % 2 == 0 else nc.scalar
                    e.dma_start(t, src2[:, fb * FBW * D:(fb + 1) * FBW * D])
                    t2.append(t)
                return t2

            def load_x(coff, N, pfx, halves=False):
                t = xpool.tile([P, DT * 512], _BF16, tag=f"x_{(coff // 512) % 2}",
                               name=f"{pfx}x")
                t = t[:, :DT * N]
                src = xp[:, xoffs[coff]:xoffs[coff] + DT * N]
                if halves:
                    h = DT * N // 2
                    nc.sync.dma_start(t[:, :h], src[:, :h])
                    nc.sync.dma_start(t[:, h:], src[:, h:])
                else:
                    nc.sync.dma_start(t, src)
                return t

            # PE warm-up: dummy matmuls on a zeroed tile (no DMA deps) keep
            # the PE busy across the DMA start so the HAM clock-gate is
            # already 8/8 (2.4 GHz) when real matmuls issue, and bridge the
            # arrival window of the critical first transfers.
            warm = bpool.tile([P, P + 512], _BF16, tag="warm", name="warm")
            nc.any.memset(warm[:, :], 0.0)
            wps = psum.tile([P, 512], _F32, tag="py", name="pwarm")
            for _ in range(WARMUP_MM):
                nc.tensor.matmul(wps, lhsT=warm[:, :P], rhs=warm[:, P:],
                                 start=True, stop=True)

            # Critical prefetch: chunk-0 X on the sync ring, Ws1 group 0 on
            # the scalar ring (parallel issue), both split in halves so the
            # d=0..3 slices land first, then biases and the bulk loads
            # (alternating rings).
            x0 = load_x(chunks[0][1], chunks[0][2], "c0_", halves=True)
            ws1_t = load_w1(ws1, "s_", [0], eng=nc.scalar, halves=True)
            b1t = bpool.tile([P, FT], _F32, tag="b1", name="b1t")
            nc.scalar.dma_start(b1t, b1r[:, :])
            bs1t = bpool.tile([P, FT], _F32, tag="bs1", name="bs1t")
            nc.scalar.dma_start(bs1t, bs1r[:, :])

            ws1_t.update(load_w1(ws1, "s_", range(1, FG)))
            ws2_t = load_w2(ws2, "s_")
            we1_t = we2_t = None

            for ci, (kind, coff, N) in enumerate(chunks):
                if kind == "s":
                    w1t, w2t, bt = ws1_t, ws2_t, bs1t
                else:
                    if we1_t is None:
                        we1_t = load_w1(w1e, "e_", range(FG))
                        we2_t = load_w2(w2e, "e_")
                    w1t, w2t, bt = we1_t, we2_t, b1t

                xc = x0 if ci == 0 else load_x(coff, N, f"c{ci}_")

                hts = []
                for f in range(FT):
                    ph = psum.tile([P, 512], _F32, tag="ph", name="ph")[:, :N]
                    g, fi = divmod(f, FT // FG)
                    for d in range(DT):
                        nc.tensor.matmul(
                            ph,
                            lhsT=w1t[g][:, d * FGW + fi * P:d * FGW + (fi + 1) * P],
                            rhs=xc[:, d * N:(d + 1) * N],
                            start=(d == 0),
                            stop=(d == DT - 1),
                        )
                    ht = hpool.tile([P, 512], _BF16, tag="h", name="h")[:, :N]
                    nc.scalar.activation(
                        ht, ph, mybir.ActivationFunctionType.Gelu,
                        bias=bt[:, f:f + 1],
                    )
                    hts.append(ht)

                for d in range(DT):
                    py = psum.tile([P, 512], _F32, tag="py", name="py")[:, :N]
                    for f in range(FT):
                        fb, ff = divmod(f, FBW)
                        nc.tensor.matmul(
                            py,
                            lhsT=w2t[fb][:, ff * D + d * P:ff * D + (d + 1) * P],
                            rhs=hts[f],
                            start=(f == 0),
                            stop=(f == FT - 1),
                        )
                    yo = ypool.tile([P, 512], _F32, tag="y", name="y")[:, :N]
                    nc.vector.tensor_copy(yo, py)
                    nc.scalar.dma_start(yt[d * P:(d + 1) * P, coff:coff + N], yo)

    nc.finalize()
    return nc


def _get_program(C: int):
    key = (C, WARMUP_MM)
    if key not in _program_cache:
        _program_cache[key] = _build_program(C)
    return _program_cache[key]


def _route(xf: np.ndarray, W_gate: np.ndarray):
    """Replicate the reference gate in float64 (selection margins are ~1e-5,
    far above fp32 rounding, so the top-2 sets match the fp32 reference)."""
    logits = xf.astype(np.float64) @ W_gate.astype(np.float64)
    m = logits.max(axis=-1, keepdims=True)
    p = np.exp(logits - m)
    p /= p.sum(axis=-1, keepdims=True)
    top_i = np.argsort(-p, axis=-1, kind="stable")[:, :TOPK]
    top_v = np.take_along_axis(p, top_i, axis=-1)
    top_v = top_v / top_v.sum(axis=-1, keepdims=True)
    return top_i, top_v.astype(np.float32)


def kernel(x, W_gate, W1, b1, W2, b2, Ws1, bs1, Ws2, bs2):
    x = np.asarray(x, np.float32)
    xf = x.reshape(T, D)
    top_i, top_v = _route(xf, np.asarray(W_gate, np.float32))

    # per-expert token lists
    idx = [np.nonzero((top_i == e).any(axis=1))[0] for e in range(E)]
    wgt = []
    for e in range(E):
        sel = top_i[idx[e]] == e  # [cnt, K] exactly one True per row
        wgt.append(top_v[idx[e]][sel].astype(np.float32))
    counts = np.array([len(i) for i in idx])
    C = int(-(-counts.max() // 8) * 8)
    NT = SHARED + C

    chunks = [(0, SHARED)]
    off = SHARED
    for w in _chunk_plan(C):
        chunks.append((off, w))
        off += w

    xbf = xf.astype(ml_dtypes.bfloat16)
    W1 = np.asarray(W1); W2 = np.asarray(W2)
    ws1_b = _pack_w1(np.asarray(Ws1, np.float32).astype(ml_dtypes.bfloat16))
    ws2_b = _pack_w2(np.asarray(Ws2, np.float32).astype(ml_dtypes.bfloat16))
    bs1r = np.ascontiguousarray(
        np.asarray(bs1, np.float32).reshape(FT, P).T)

    in_maps = []
    for c in range(E):
        pad_idx = np.zeros(C, np.int64)
        pad_idx[:counts[c]] = idx[c]
        xcols = np.concatenate([xbf[c * SHARED:(c + 1) * SHARED], xbf[pad_idx]], axis=0)
        in_maps.append({
            "xp": _pack_x(xcols, chunks),
            "w1e": _pack_w1(np.asarray(W1[c], np.float32).astype(ml_dtypes.bfloat16)),
            "w2e": _pack_w2(np.asarray(W2[c], np.float32).astype(ml_dtypes.bfloat16)),
            "ws1": ws1_b,
            "ws2": ws2_b,
            "b1r": np.ascontiguousarray(np.asarray(b1[c], np.float32).reshape(FT, P).T),
            "bs1r": bs1r,
        })

    nc = _get_program(C)
    global last_results
    last_results = run_bass_kernel_spmd(
        nc, in_maps, list(range(NCORES)), **TRACE_KWARGS)
    res = last_results.results

    out = np.zeros((T, D), np.float32)
    for c in range(E):
        y = np.asarray(res[c]["yt"], np.float32)
        out[c * SHARED:(c + 1) * SHARED] += y[:, :SHARED].T
        cnt = counts[c]
        out[idx[c]] += wgt[c][:, None] * y[:, SHARED:SHARED + cnt].T

    # biases enter linearly; add on host (zeros in this problem's inputs)
    b2 = np.asarray(b2, np.float32)
    bs2 = np.asarray(bs2, np.float32)
    combine = np.zeros((T, E), np.float32)
    np.put_along_axis(combine, top_i, top_v, axis=1)
    out += combine @ b2 + bs2

    return out.reshape(B, L, D)


# revision 13
# speedup vs baseline: 1.2118x; 1.2118x over previous
"""MoE block (8 experts, top-2, + shared expert) on 8 trn2 NeuronCores.

Strategy (expert-parallel, host dispatch):
  - Host computes gate logits/softmax/top-2 (0.03% of total FLOPs) and
    dispatches tokens: core c receives the tokens routed to expert c
    (padded to the max per-expert count) plus a 1/8 slice of all tokens
    for the shared expert.
  - Each core runs two FFN passes in one Bass program: shared FFN on its
    512-token slice, then expert-c FFN on its routed tokens. Matmuls are
    bf16 (weights + activations) with fp32 PSUM accumulation; feature-major
    ([D, tokens]) layout avoids all on-device transposes.
  - Host combines: routed outputs scaled by renormalized top-2 weights and
    scatter-added, shared outputs added per-slice, biases b2/bs2 added on
    host (they enter linearly).

DMA layout: all weight/x tensors are host-packed so every transfer has
>=8KB contiguous per partition (1KB strided lines measured ~11 GB/s per
queue; 8KB lines approach the 341 GB/s 1MB-transfer rate):
  w1p [128, 32768]: cols (g, d, j) so one per-g transfer [128, 4096]
      yields lhsT slices [:, d*512+fi*128 :+128] (partition = contraction).
  w2p [128, 32768]: cols (fb, ff, j) so one per-fb transfer [128, 8192]
      yields lhsT slices [:, ff*1024+d*128 :+128].
  xp  [128, 8*NT]: per chunk cols (d, t) so one transfer per chunk.
Each dma_start instruction costs ~600ns serialized on its HWDGE ring, so
the startup-critical pair (x chunk 0 + shared-w1 g0) is issued as just two
transfers, one on each of the two HWDGE rings (sync + scalar) so they
issue in parallel; biases and y-output stores also go to the scalar ring
to keep the sync ring free for bulk weight streaming.
"""

import numpy as np
import ml_dtypes

import concourse.bass as bass
import concourse.bacc as bacc
from concourse import mybir
from concourse.tile import TileContext
from concourse.bass_utils import run_bass_kernel_spmd

D = 1024
FF = 4096
E = 8
TOPK = 2
B, L = 4, 1024
T = B * L
NCORES = 8
SHARED = T // NCORES  # shared-expert tokens per core
P = 128
DT = D // P    # 8 k-tiles for D
FT = FF // P   # 32 tiles for FF
FG = 8         # w1 f-groups (one transfer each)
FGW = FF // FG # 512 w1 cols per f-group
FB = 4         # w2 f-blocks (one transfer each)
FBW = FT // FB # 8 f-tiles per w2 block

_BF16 = mybir.dt.bfloat16
_F32 = mybir.dt.float32

_program_cache: dict[tuple, tuple] = {}

# test harness hooks: extra kwargs for run_bass_kernel_spmd (e.g. trace=True)
# and the last BassKernelResults for profiling. Unused in normal grading runs.
TRACE_KWARGS: dict = {}
last_results = None

WARMUP_MM = 28


def _chunk_plan(C: int) -> list[int]:
    """Split C routed columns into <=512-wide chunks, sizes multiple of 8."""
    n = -(-C // 512)
    base = -(-C // n)
    base = -(-base // 8) * 8
    sizes = []
    left = C
    for _ in range(n - 1):
        sizes.append(base)
        left -= base
    sizes.append(left)
    assert all(0 < s <= 512 for s in sizes) and sum(sizes) == C
    return sizes


def _pack_w1(W: np.ndarray) -> np.ndarray:
    """[D, FF] -> [128, FG*DT*FGW] with cols (g, d, j)."""
    return np.ascontiguousarray(
        W.reshape(DT, P, FG, FGW).transpose(1, 2, 0, 3).reshape(P, FG * DT * FGW)
    )


def _pack_w2(W: np.ndarray) -> np.ndarray:
    """[FF, D] -> [128, FB*FBW*D] with cols (fb, ff, j)."""
    return np.ascontiguousarray(
        W.reshape(FB, FBW, P, D).transpose(2, 0, 1, 3).reshape(P, FB * FBW * D)
    )


def _pack_x(xcols: np.ndarray, chunks: list[tuple[int, int]]) -> np.ndarray:
    """[NT, D] bf16 -> [128, DT*NT] with per-chunk col blocks (d, t)."""
    blocks = []
    for off, N in chunks:
        blk = xcols[off:off + N].reshape(N, DT, P).transpose(2, 1, 0)
        blocks.append(blk.reshape(P, DT * N))
    return np.ascontiguousarray(np.concatenate(blocks, axis=1))


def _build_program(C: int):
    """One SPMD Bass program: shared FFN (512 cols) + expert FFN (C cols)."""
    NT = SHARED + C
    nc = bacc.Bacc()

    xp = nc.dram_tensor("xp", [P, DT * NT], _BF16, kind="ExternalInput")
    w1e = nc.dram_tensor("w1e", [P, FG * DT * FGW], _BF16, kind="ExternalInput")
    w2e = nc.dram_tensor("w2e", [P, FB * FBW * D], _BF16, kind="ExternalInput")
    ws1 = nc.dram_tensor("ws1", [P, FG * DT * FGW], _BF16, kind="ExternalInput")
    ws2 = nc.dram_tensor("ws2", [P, FB * FBW * D], _BF16, kind="ExternalInput")
    b1r = nc.dram_tensor("b1r", [P, FT], _F32, kind="ExternalInput")
    bs1r = nc.dram_tensor("bs1r", [P, FT], _F32, kind="ExternalInput")
    yt = nc.dram_tensor("yt", [D, NT], _F32, kind="ExternalOutput")

    # chunks: (weights_key, col_offset, width); shared phase first so the
    # expert weights can stream in (reusing the same SBUF slots) while the
    # shared phase computes.
    chunks = [("s", 0, SHARED)]
    off = SHARED
    for w in _chunk_plan(C):
        chunks.append(("e", off, w))
        off += w
    # packed-x col offset of each chunk: DT * token-offset
    xoffs = {}
    o = 0
    for _, coff, N in chunks:
        xoffs[coff] = o
        o += DT * N

    with TileContext(nc) as tc:
        with (
            tc.tile_pool(name="wpool", bufs=1) as wpool,
            tc.tile_pool(name="xpool", bufs=1) as xpool,
            tc.tile_pool(name="hpool", bufs=34) as hpool,
            tc.tile_pool(name="ypool", bufs=8) as ypool,
            tc.tile_pool(name="bpool", bufs=1) as bpool,
            tc.tile_pool(name="psum", bufs=4, space="PSUM") as psum,
        ):
            def load_w1(src1, pfx, groups, eng=None, halves=False):
                t1 = {}
                for g in groups:
                    t = wpool.tile([P, DT * FGW], _BF16, tag=f"w1_{g}",
                                   name=f"{pfx}w1_{g}")
                    src = src1[:, g * DT * FGW:(g + 1) * DT * FGW]
                    e = eng or nc.sync
                    if halves:
                        h = DT * FGW // 2
                        e.dma_start(t[:, :h], src[:, :h])
                        e.dma_start(t[:, h:], src[:, h:])
                    else:
                        e.dma_start(t, src)
                    t1[g] = t
                return t1

            def load_w2(src2, pfx):
                t2 = []
                for fb in range(FB):
                    t = wpool.tile([P, FBW * D], _BF16, tag=f"w2_{fb}",
                                   name=f"{pfx}w2_{fb}")
                    nc.sync.dma_start(t, src2[:, fb * FBW * D:(fb + 1) * FBW * D])
                    t2.append(t)
                return t2

            def load_x(coff, N, pfx, halves=False):
                t = xpool.tile([P, DT * 512], _BF16, tag=f"x_{(coff // 512) % 2}",
                               name=f"{pfx}x")
                t = t[:, :DT * N]
                src = xp[:, xoffs[coff]:xoffs[coff] + DT * N]
                if halves:
                    h = DT * N // 2
                    nc.sync.dma_start(t[:, :h], src[:, :h])
                    nc.sync.dma_start(t[:, h:], src[:, h:])
                else:
                    nc.sync.dma_start(t, src)
                return t

            # PE warm-up: dummy matmuls on a zeroed tile (no DMA deps) keep
            # the PE busy across the DMA start so the HAM clock-gate is
            # already 8/8 (2.4 GHz) when real matmuls issue, and bridge the
            # arrival window of the critical first transfers.
            warm = bpool.tile([P, P + 512], _BF16, tag="warm", name="warm")
            nc.any.memset(warm[:, :], 0.0)
            wps = psum.tile([P, 512], _F32, tag="py", name="pwarm")
            for _ in range(WARMUP_MM):
                nc.tensor.matmul(wps, lhsT=warm[:, :P], rhs=warm[:, P:],
                                 start=True, stop=True)

            # Critical prefetch: chunk-0 X on the sync ring, Ws1 group 0 on
            # the scalar ring (parallel issue), both split in halves so the
            # d=0..3 slices land first, then biases and the bulk loads
            # (alternating rings).
            x0 = load_x(chunks[0][1], chunks[0][2], "c0_", halves=True)
            ws1_t = load_w1(ws1, "s_", [0], eng=nc.scalar, halves=True)
            b1t = bpool.tile([P, FT], _F32, tag="b1", name="b1t")
            nc.scalar.dma_start(b1t, b1r[:, :])
            bs1t = bpool.tile([P, FT], _F32, tag="bs1", name="bs1t")
            nc.scalar.dma_start(bs1t, bs1r[:, :])

            ws1_t.update(load_w1(ws1, "s_", range(1, FG)))
            ws2_t = load_w2(ws2, "s_")
            we1_t = we2_t = None

            for ci, (kind, coff, N) in enumerate(chunks):
                if kind == "s":
                    w1t, w2t, bt = ws1_t, ws2_t, bs1t
                else:
                    if we1_t is None:
                        we1_t = load_w1(w1e, "e_", range(FG))
                        we2_t = load_w2(w2e, "e_")
                    w1t, w2t, bt = we1_t, we2_t, b1t

                xc = x0 if ci == 0 else load_x(coff, N, f"c{ci}_")

                hts = []
                for f in range(FT):
                    ph = psum.tile([P, 512], _F32, tag="ph", name="ph")[:, :N]
                    g, fi = divmod(f, FT // FG)
                    for d in range(DT):
                        nc.tensor.matmul(
                            ph,
                            lhsT=w1t[g][:, d * FGW + fi * P:d * FGW + (fi + 1) * P],
                            rhs=xc[:, d * N:(d + 1) * N],
                            start=(d == 0),
                            stop=(d == DT - 1),
                        )
                    ht = hpool.tile([P, 512], _BF16, tag="h", name="h")[:, :N]
                    nc.scalar.activation(
                        ht, ph, mybir.ActivationFunctionType.Gelu,
                        bias=bt[:, f:f + 1],
                    )
                    hts.append(ht)

                for d in range(DT):
                    py = psum.tile([P, 512], _F32, tag="py", name="py")[:, :N]
                    for f in range(FT):
                        fb, ff = divmod(f, FBW)
                        nc.tensor.matmul(
                            py,
                            lhsT=w2t[fb][:, ff * D + d * P:ff * D + (d + 1) * P],
                            rhs=hts[f],
                            start=(f == 0),
                            stop=(f == FT - 1),
                        )
                    yo = ypool.tile([P, 512], _F32, tag="y", name="y")[:, :N]
                    nc.vector.tensor_copy(yo, py)
                    nc.scalar.dma_start(yt[d * P:(d + 1) * P, coff:coff + N], yo)

    nc.finalize()
    return nc


def _get_program(C: int):
    key = (C, WARMUP_MM)
    if key not in _program_cache:
        _program_cache[key] = _build_program(C)
    return _program_cache[key]


def _route(xf: np.ndarray, W_gate: np.ndarray):
    """Replicate the reference gate in float64 (selection margins are ~1e-5,
    far above fp32 rounding, so the top-2 sets match the fp32 reference)."""
    logits = xf.astype(np.float64) @ W_gate.astype(np.float64)
    m = logits.max(axis=-1, keepdims=True)
    p = np.exp(logits - m)
    p /= p.sum(axis=-1, keepdims=True)
    top_i = np.argsort(-p, axis=-1, kind="stable")[:, :TOPK]
    top_v = np.take_along_axis(p, top_i, axis=-1)
    top_v = top_v / top_v.sum(axis=-1, keepdims=True)
    return top_i, top_v.astype(np.float32)


def kernel(x, W_gate, W1, b1, W2, b2, Ws1, bs1, Ws2, bs2):
    x = np.asarray(x, np.float32)
    xf = x.reshape(T, D)
    top_i, top_v = _route(xf, np.asarray(W_gate, np.float32))

    # per-expert token lists
    idx = [np.nonzero((top_i == e).any(axis=1))[0] for e in range(E)]
    wgt = []
    for e in range(E):
        sel = top_i[idx[e]] == e  # [cnt, K] exactly one True per row
        wgt.append(top_v[idx[e]][sel].astype(np.float32))
    counts = np.array([len(i) for i in idx])
    C = int(-(-counts.max() // 8) * 8)
    NT = SHARED + C

    chunks = [(0, SHARED)]
    off = SHARED
    for w in _chunk_plan(C):
        chunks.append((off, w))
        off += w

    xbf = xf.astype(ml_dtypes.bfloat16)
    W1 = np.asarray(W1); W2 = np.asarray(W2)
    ws1_b = _pack_w1(np.asarray(Ws1, np.float32).astype(ml_dtypes.bfloat16))
    ws2_b = _pack_w2(np.asarray(Ws2, np.float32).astype(ml_dtypes.bfloat16))
    bs1r = np.ascontiguousarray(
        np.asarray(bs1, np.float32).reshape(FT, P).T)

    in_maps = []
    for c in range(E):
        pad_idx = np.zeros(C, np.int64)
        pad_idx[:counts[c]] = idx[c]
        xcols = np.concatenate([xbf[c * SHARED:(c + 1) * SHARED], xbf[pad_idx]], axis=0)
        in_maps.append({
            "xp": _pack_x(xcols, chunks),
            "w1e": _pack_w1(np.asarray(W1[c], np.float32).astype(ml_dtypes.bfloat16)),
            "w2e": _pack_w2(np.asarray(W2[c], np.float32).astype(ml_dtypes.bfloat16)),
            "ws1": ws1_b,
            "ws2": ws2_b,
            "b1r": np.ascontiguousarray(np.asarray(b1[c], np.float32).reshape(FT, P).T),
            "bs1r": bs1r,
        })

    nc = _get_program(C)
    global last_results
    last_results = run_bass_kernel_spmd(
        nc, in_maps, list(range(NCORES)), **TRACE_KWARGS)
    res = last_results.results

    out = np.zeros((T, D), np.float32)
    for c in range(E):
        y = np.asarray(res[c]["yt"], np.float32)
        out[c * SHARED:(c + 1) * SHARED] += y[:, :SHARED].T
        cnt = counts[c]
        out[idx[c]] += wgt[c][:, None] * y[:, SHARED:SHARED + cnt].T

    # biases enter linearly; add on host (zeros in this problem's inputs)
    b2 = np.asarray(b2, np.float32)
    bs2 = np.asarray(bs2, np.float32)
    combine = np.zeros((T, E), np.float32)
    np.put_along_axis(combine, top_i, top_v, axis=1)
    out += combine @ b2 + bs2

    return out.reshape(B, L, D)


# revision 14
# speedup vs baseline: 1.2383x; 1.0219x over previous
"""MoE block (8 experts, top-2, + shared expert) on 8 trn2 NeuronCores.

Strategy (expert-parallel with pairing, host dispatch):
  - Host computes gate logits/softmax/top-2 (0.03% of total FLOPs) and
    dispatches tokens. To balance load, each core runs THREE FFN weight
    sets: the shared expert on 512 tokens, plus two expert "cells" (slot A,
    slot B) with uniform widths wA/wB across cores. The two largest experts
    each occupy two A-cells (split ~50/50), the two smallest two B-cells,
    and the middle four one A-cell + one B-cell each. This gives every core
    512+wA+wB ~= 1568 token-passes instead of 512+max_count ~= 1608.
  - Matmuls are bf16 (weights + activations) with fp32 PSUM accumulation;
    feature-major ([D, tokens]) layout avoids all on-device transposes.
  - Host combines: routed outputs scaled by renormalized top-2 weights and
    scatter-added, shared outputs added per-slice, biases b2/bs2 added on
    host (they enter linearly).

DMA layout: all weight/x tensors are host-packed so every transfer has
>=8KB contiguous per partition (1KB strided lines measured ~11 GB/s per
queue; 8KB lines approach the 341 GB/s 1MB-transfer rate):
  w1 packed [128, 32768]: cols (g, d, j) so one per-g transfer [128, 4096]
      yields lhsT slices [:, d*512+fi*128 :+128] (partition = contraction).
  w2 packed [128, 32768]: cols (fb, ff, j) so one per-fb transfer
      [128, 8192] yields lhsT slices [:, ff*1024+d*128 :+128].
  xp [128, 8*NT]: per chunk cols (d, t) so one transfer per chunk.
Each dma_start instruction costs ~600ns serialized on its HWDGE ring, so
the startup-critical pair (x chunk 0 + shared-w1 g0) issues as two
half-transfers on each of the two HWDGE rings (sync + scalar, parallel
issue; the first halves cover the d=0..3 matmuls). Biases and y-output
stores also use the scalar ring; bulk weight streaming owns the sync ring
(bulk on the scalar ring measured badly: its issues queue behind gelu
ACTIVATEs in the ACT engine's strict FIFO and stall the PE).

Weight-set streaming order is [shared | B | A]: the B set streams during
the long shared phase, the A set during the B phase, reusing the same
SBUF slots (tag rotation provides the pacing dependencies).
"""

import numpy as np
import ml_dtypes

import concourse.bass as bass
import concourse.bacc as bacc
from concourse import mybir
from concourse.tile import TileContext
from concourse.bass_utils import run_bass_kernel_spmd

D = 1024
FF = 4096
E = 8
TOPK = 2
B, L = 4, 1024
T = B * L
NCORES = 8
SHARED = T // NCORES  # shared-expert tokens per core
P = 128
DT = D // P    # 8 k-tiles for D
FT = FF // P   # 32 tiles for FF
FG = 8         # w1 f-groups (one transfer each)
FGW = FF // FG # 512 w1 cols per f-group
FB = 4         # w2 f-blocks (one transfer each)
FBW = FT // FB # 8 f-tiles per w2 block

_BF16 = mybir.dt.bfloat16
_F32 = mybir.dt.float32

_program_cache: dict[tuple, tuple] = {}

# test harness hooks: extra kwargs for run_bass_kernel_spmd (e.g. trace=True)
# and the last BassKernelResults for profiling. Unused in normal grading runs.
TRACE_KWARGS: dict = {}
last_results = None

WARMUP_MM = 28


def _ru8(v: int) -> int:
    return -(-int(v) // 8) * 8


def _split_chunks(w: int) -> list[int]:
    """Split a slot width into <=512-wide chunk widths, multiples of 8."""
    if w <= 512:
        return [w]
    n = -(-w // 512)
    base = _ru8(-(-w // n))
    sizes = [base] * (n - 1)
    sizes.append(w - base * (n - 1))
    assert all(0 < s <= 512 and s % 8 == 0 for s in sizes)
    return sizes


def _pack_w1(W: np.ndarray) -> np.ndarray:
    """[D, FF] -> [128, FG*DT*FGW] with cols (g, d, j)."""
    return np.ascontiguousarray(
        W.reshape(DT, P, FG, FGW).transpose(1, 2, 0, 3).reshape(P, FG * DT * FGW)
    )


def _pack_w2(W: np.ndarray) -> np.ndarray:
    """[FF, D] -> [128, FB*FBW*D] with cols (fb, ff, j)."""
    return np.ascontiguousarray(
        W.reshape(FB, FBW, P, D).transpose(2, 0, 1, 3).reshape(P, FB * FBW * D)
    )


def _pack_x(xcols: np.ndarray, widths: list[int]) -> np.ndarray:
    """[NT, D] bf16 -> [128, DT*NT] with per-chunk col blocks (d, t)."""
    blocks = []
    off = 0
    for N in widths:
        blk = xcols[off:off + N].reshape(N, DT, P).transpose(2, 1, 0)
        blocks.append(blk.reshape(P, DT * N))
        off += N
    return np.ascontiguousarray(np.concatenate(blocks, axis=1))


def _build_program(wA: int, wB: int):
    """One SPMD Bass program: shared FFN + B-slot FFN + A-slot FFN."""
    NT = SHARED + wB + wA
    nc = bacc.Bacc()

    xp = nc.dram_tensor("xp", [P, DT * NT], _BF16, kind="ExternalInput")
    w1a = nc.dram_tensor("w1a", [P, FG * DT * FGW], _BF16, kind="ExternalInput")
    w2a = nc.dram_tensor("w2a", [P, FB * FBW * D], _BF16, kind="ExternalInput")
    w1b = nc.dram_tensor("w1b", [P, FG * DT * FGW], _BF16, kind="ExternalInput")
    w2b = nc.dram_tensor("w2b", [P, FB * FBW * D], _BF16, kind="ExternalInput")
    ws1 = nc.dram_tensor("ws1", [P, FG * DT * FGW], _BF16, kind="ExternalInput")
    ws2 = nc.dram_tensor("ws2", [P, FB * FBW * D], _BF16, kind="ExternalInput")
    b1a = nc.dram_tensor("b1a", [P, FT], _F32, kind="ExternalInput")
    b1b = nc.dram_tensor("b1b", [P, FT], _F32, kind="ExternalInput")
    bs1r = nc.dram_tensor("bs1r", [P, FT], _F32, kind="ExternalInput")
    yt = nc.dram_tensor("yt", [D, NT], _F32, kind="ExternalOutput")

    # chunks: (set_key, col_offset, width); shared first, then B, then A
    # (each later set's weights stream in during the previous phases).
    chunks = [("s", 0, SHARED)]
    off = SHARED
    for w in _split_chunks(wB):
        chunks.append(("b", off, w))
        off += w
    for w in _split_chunks(wA):
        chunks.append(("a", off, w))
        off += w
    # packed-x col offset of each chunk
    xoffs = {}
    o = 0
    for _, coff, N in chunks:
        xoffs[coff] = o
        o += DT * N

    with TileContext(nc) as tc:
        with (
            tc.tile_pool(name="wpool", bufs=1) as wpool,
            tc.tile_pool(name="xpool", bufs=1) as xpool,
            tc.tile_pool(name="hpool", bufs=34) as hpool,
            tc.tile_pool(name="ypool", bufs=8) as ypool,
            tc.tile_pool(name="bpool", bufs=1) as bpool,
            tc.tile_pool(name="psum", bufs=4, space="PSUM") as psum,
        ):
            def load_w1(src1, pfx, groups, eng=None, halves=False):
                t1 = {}
                for g in groups:
                    t = wpool.tile([P, DT * FGW], _BF16, tag=f"w1_{g}",
                                   name=f"{pfx}w1_{g}")
                    src = src1[:, g * DT * FGW:(g + 1) * DT * FGW]
                    e = eng or nc.sync
                    if halves:
                        h = DT * FGW // 2
                        e.dma_start(t[:, :h], src[:, :h])
                        e.dma_start(t[:, h:], src[:, h:])
                    else:
                        e.dma_start(t, src)
                    t1[g] = t
                return t1

            def load_w2(src2, pfx):
                t2 = []
                for fb in range(FB):
                    t = wpool.tile([P, FBW * D], _BF16, tag=f"w2_{fb}",
                                   name=f"{pfx}w2_{fb}")
                    nc.sync.dma_start(t, src2[:, fb * FBW * D:(fb + 1) * FBW * D])
                    t2.append(t)
                return t2

            def load_x(coff, N, pfx, halves=False):
                t = xpool.tile([P, DT * 512], _BF16, tag=f"x_{(coff // 512) % 2}",
                               name=f"{pfx}x")
                t = t[:, :DT * N]
                src = xp[:, xoffs[coff]:xoffs[coff] + DT * N]
                if halves:
                    h = DT * N // 2
                    nc.sync.dma_start(t[:, :h], src[:, :h])
                    nc.sync.dma_start(t[:, h:], src[:, h:])
                else:
                    nc.sync.dma_start(t, src)
                return t

            # PE warm-up: dummy matmuls on a zeroed tile (no DMA deps) keep
            # the PE busy across the DMA start so the HAM clock-gate is
            # already 8/8 (2.4 GHz) when real matmuls issue, and bridge the
            # arrival window of the critical first transfers.
            warm = bpool.tile([P, P + 512], _BF16, tag="warm", name="warm")
            nc.any.memset(warm[:, :], 0.0)
            wps = psum.tile([P, 512], _F32, tag="py", name="pwarm")
            for _ in range(WARMUP_MM):
                nc.tensor.matmul(wps, lhsT=warm[:, :P], rhs=warm[:, P:],
                                 start=True, stop=True)

            # Critical prefetch: chunk-0 X on the sync ring, Ws1 group 0 on
            # the scalar ring (parallel issue), both split in halves so the
            # d=0..3 slices land first, then biases and the bulk loads.
            x0 = load_x(chunks[0][1], chunks[0][2], "c0_", halves=True)
            ws1_t = load_w1(ws1, "s_", [0], eng=nc.scalar, halves=True)
            bs1t = bpool.tile([P, FT], _F32, tag="bs1", name="bs1t")
            nc.scalar.dma_start(bs1t, bs1r[:, :])
            b1bt = bpool.tile([P, FT], _F32, tag="b1b", name="b1bt")
            nc.scalar.dma_start(b1bt, b1b[:, :])
            b1at = bpool.tile([P, FT], _F32, tag="b1a", name="b1at")
            nc.scalar.dma_start(b1at, b1a[:, :])

            ws1_t.update(load_w1(ws1, "s_", range(1, FG)))
            ws2_t = load_w2(ws2, "s_")
            wb_loaded = wa_loaded = None

            for ci, (kind, coff, N) in enumerate(chunks):
                if kind == "s":
                    w1t, w2t, bt = ws1_t, ws2_t, bs1t
                elif kind == "b":
                    if wb_loaded is None:
                        wb1_t = load_w1(w1b, "b_", range(FG))
                        wb2_t = load_w2(w2b, "b_")
                        wb_loaded = True
                    w1t, w2t, bt = wb1_t, wb2_t, b1bt
                else:
                    if wa_loaded is None:
                        wa1_t = load_w1(w1a, "a_", range(FG))
                        wa2_t = load_w2(w2a, "a_")
                        wa_loaded = True
                    w1t, w2t, bt = wa1_t, wa2_t, b1at

                xc = x0 if ci == 0 else load_x(coff, N, f"c{ci}_")

                hts = []
                for f in range(FT):
                    ph = psum.tile([P, 512], _F32, tag="ph", name="ph")[:, :N]
                    g, fi = divmod(f, FT // FG)
                    for d in range(DT):
                        nc.tensor.matmul(
                            ph,
                            lhsT=w1t[g][:, d * FGW + fi * P:d * FGW + (fi + 1) * P],
                            rhs=xc[:, d * N:(d + 1) * N],
                            start=(d == 0),
                            stop=(d == DT - 1),
                        )
                    ht = hpool.tile([P, 512], _BF16, tag="h", name="h")[:, :N]
                    nc.scalar.activation(
                        ht, ph, mybir.ActivationFunctionType.Gelu,
                        bias=bt[:, f:f + 1],
                    )
                    hts.append(ht)

                for d in range(DT):
                    py = psum.tile([P, 512], _F32, tag="py", name="py")[:, :N]
                    for f in range(FT):
                        fb, ff = divmod(f, FBW)
                        nc.tensor.matmul(
                            py,
                            lhsT=w2t[fb][:, ff * D + d * P:ff * D + (d + 1) * P],
                            rhs=hts[f],
                            start=(f == 0),
                            stop=(f == FT - 1),
                        )
                    yo = ypool.tile([P, 512], _F32, tag="y", name="y")[:, :N]
                    nc.vector.tensor_copy(yo, py)
                    nc.scalar.dma_start(yt[d * P:(d + 1) * P, coff:coff + N], yo)

    nc.finalize()
    return nc


def _get_program(wA: int, wB: int):
    key = (wA, wB, WARMUP_MM)
    if key not in _program_cache:
        _program_cache[key] = _build_program(wA, wB)
    return _program_cache[key]


def _route(xf: np.ndarray, W_gate: np.ndarray):
    """Replicate the reference gate in float64 (selection margins are ~1e-5,
    far above fp32 rounding, so the top-2 sets match the fp32 reference)."""
    logits = xf.astype(np.float64) @ W_gate.astype(np.float64)
    m = logits.max(axis=-1, keepdims=True)
    p = np.exp(logits - m)
    p /= p.sum(axis=-1, keepdims=True)
    top_i = np.argsort(-p, axis=-1, kind="stable")[:, :TOPK]
    top_v = np.take_along_axis(p, top_i, axis=-1)
    top_v = top_v / top_v.sum(axis=-1, keepdims=True)
    return top_i, top_v.astype(np.float32)


def _plan_cells(counts: np.ndarray):
    """Assign experts to 8 A-cells (width wA) and 8 B-cells (width wB).

    Returns (wA, wB, cells) where cells[core] = (cellA, cellB) and each
    cell = (expert, tok_start, tok_count) within the expert's token list.
    """
    order = np.argsort(-counts, kind="stable")
    big2, mid4, small2 = order[:2], order[2:6], order[6:]
    wA = _ru8(-(-int(counts[big2[0]]) // 2))
    wB = _ru8(-(-int(counts[small2[0]]) // 2))
    need = int(counts[mid4].max()) if len(mid4) else 0
    if wA + wB < need:
        wB = _ru8(need - wA)

    cellsA, cellsB = [], []
    for e in big2:
        cnt = int(counts[e])
        h = cnt // 2
        cellsA.append((int(e), 0, h))
        cellsA.append((int(e), h, cnt - h))
    for e in small2:
        cnt = int(counts[e])
        h = cnt // 2
        cellsB.append((int(e), 0, h))
        cellsB.append((int(e), h, cnt - h))
    for e in mid4:
        cnt = int(counts[e])
        s = min(wA, cnt)
        cellsA.append((int(e), 0, s))
        cellsB.append((int(e), s, cnt - s))
    assert len(cellsA) == 8 and len(cellsB) == 8
    for e, st, cn in cellsA:
        assert cn <= wA
    for e, st, cn in cellsB:
        assert cn <= wB
    return wA, wB, list(zip(cellsA, cellsB))


def kernel(x, W_gate, W1, b1, W2, b2, Ws1, bs1, Ws2, bs2):
    x = np.asarray(x, np.float32)
    xf = x.reshape(T, D)
    top_i, top_v = _route(xf, np.asarray(W_gate, np.float32))

    # per-expert token lists
    idx = [np.nonzero((top_i == e).any(axis=1))[0] for e in range(E)]
    wgt = []
    for e in range(E):
        sel = top_i[idx[e]] == e  # [cnt, K] exactly one True per row
        wgt.append(top_v[idx[e]][sel].astype(np.float32))
    counts = np.array([len(i) for i in idx])

    wA, wB, cells = _plan_cells(counts)
    NT = SHARED + wB + wA
    widths = [SHARED] + _split_chunks(wB) + _split_chunks(wA)

    xbf = xf.astype(ml_dtypes.bfloat16)
    W1 = np.asarray(W1); W2 = np.asarray(W2)
    b1 = np.asarray(b1, np.float32)
    ws1_b = _pack_w1(np.asarray(Ws1, np.float32).astype(ml_dtypes.bfloat16))
    ws2_b = _pack_w2(np.asarray(Ws2, np.float32).astype(ml_dtypes.bfloat16))
    bs1r = np.ascontiguousarray(
        np.asarray(bs1, np.float32).reshape(FT, P).T)
    w1_b = [_pack_w1(W1[e].astype(np.float32).astype(ml_dtypes.bfloat16))
            for e in range(E)]
    w2_b = [_pack_w2(W2[e].astype(np.float32).astype(ml_dtypes.bfloat16))
            for e in range(E)]
    b1_r = [np.ascontiguousarray(b1[e].reshape(FT, P).T) for e in range(E)]

    in_maps = []
    for c in range(NCORES):
        (eA, sA, nA), (eB, sB, nB) = cells[c]
        padA = np.zeros(wA, np.int64)
        padA[:nA] = idx[eA][sA:sA + nA]
        padB = np.zeros(wB, np.int64)
        padB[:nB] = idx[eB][sB:sB + nB]
        xcols = np.concatenate(
            [xbf[c * SHARED:(c + 1) * SHARED], xbf[padB], xbf[padA]], axis=0)
        in_maps.append({
            "xp": _pack_x(xcols, widths),
            "w1a": w1_b[eA], "w2a": w2_b[eA], "b1a": b1_r[eA],
            "w1b": w1_b[eB], "w2b": w2_b[eB], "b1b": b1_r[eB],
            "ws1": ws1_b, "ws2": ws2_b, "bs1r": bs1r,
        })

    nc = _get_program(wA, wB)
    global last_results
    last_results = run_bass_kernel_spmd(
        nc, in_maps, list(range(NCORES)), **TRACE_KWARGS)
    res = last_results.results

    out = np.zeros((T, D), np.float32)
    for c in range(NCORES):
        (eA, sA, nA), (eB, sB, nB) = cells[c]
        y = np.asarray(res[c]["yt"], np.float32)
        out[c * SHARED:(c + 1) * SHARED] += y[:, :SHARED].T
        tb = idx[eB][sB:sB + nB]
        out[tb] += wgt[eB][sB:sB + nB, None] * y[:, SHARED:SHARED + nB].T
        ta = idx[eA][sA:sA + nA]
        out[ta] += wgt[eA][sA:sA + nA, None] * \
            y[:, SHARED + wB:SHARED + wB + nA].T

    # biases enter linearly; add on host (zeros in this problem's inputs)
    b2 = np.asarray(b2, np.float32)
    bs2 = np.asarray(bs2, np.float32)
    combine = np.zeros((T, E), np.float32)
    np.put_along_axis(combine, top_i, top_v, axis=1)
    out += combine @ b2 + bs2

    return out.reshape(B, L, D)


# revision 15
# speedup vs baseline: 1.2445x; 1.0050x over previous
"""MoE block (8 experts, top-2, + shared expert) on 8 trn2 NeuronCores.

Strategy (expert-parallel with pairing, host dispatch):
  - Host computes gate logits/softmax/top-2 (0.03% of total FLOPs) and
    dispatches tokens. To balance load, each core runs THREE FFN weight
    sets: the shared expert on 512 tokens, plus two expert "cells" (slot A,
    slot B) with uniform widths wA/wB across cores. The two largest experts
    each occupy two A-cells (split ~50/50), the two smallest two B-cells,
    and the middle four one A-cell + one B-cell each. This gives every core
    512+wA+wB ~= 1568 token-passes instead of 512+max_count ~= 1608.
  - Matmuls are bf16 (weights + activations) with fp32 PSUM accumulation;
    feature-major ([D, tokens]) layout avoids all on-device transposes.
  - Host combines: routed outputs scaled by renormalized top-2 weights and
    scatter-added, shared outputs added per-slice, biases b2/bs2 added on
    host (they enter linearly).

DMA layout: all weight/x tensors are host-packed so every transfer has
>=8KB contiguous per partition (1KB strided lines measured ~11 GB/s per
queue; 8KB lines approach the 341 GB/s 1MB-transfer rate):
  w1 packed [128, 32768]: cols (g, d, j) so one per-g transfer [128, 4096]
      yields lhsT slices [:, d*512+fi*128 :+128] (partition = contraction).
  w2 packed [128, 32768]: cols (fb, ff, j) so one per-fb transfer
      [128, 8192] yields lhsT slices [:, ff*1024+d*128 :+128].
  xp [128, 8*NT]: per chunk cols (d, t) so one transfer per chunk.
Each dma_start instruction costs ~600ns serialized on its HWDGE ring, so
the startup-critical pair (x chunk 0 + shared-w1 g0) issues as two
half-transfers on each of the two HWDGE rings (sync + scalar, parallel
issue; the first halves cover the d=0..3 matmuls). Biases and y-output
stores also use the scalar ring; bulk weight streaming owns the sync ring
(bulk on the scalar ring measured badly: its issues queue behind gelu
ACTIVATEs in the ACT engine's strict FIFO and stall the PE).

Weight-set streaming order is [shared | B | A]: the B set streams during
the long shared phase, the A set during the B phase, reusing the same
SBUF slots (tag rotation provides the pacing dependencies).
"""

import numpy as np
import ml_dtypes

import concourse.bass as bass
import concourse.bacc as bacc
from concourse import mybir
from concourse.tile import TileContext
from concourse.bass_utils import run_bass_kernel_spmd

D = 1024
FF = 4096
E = 8
TOPK = 2
B, L = 4, 1024
T = B * L
NCORES = 8
SHARED = T // NCORES  # shared-expert tokens per core
P = 128
DT = D // P    # 8 k-tiles for D
FT = FF // P   # 32 tiles for FF
FG = 8         # w1 f-groups (one transfer each)
FGW = FF // FG # 512 w1 cols per f-group
FB = 4         # w2 f-blocks (one transfer each)
FBW = FT // FB # 8 f-tiles per w2 block

_BF16 = mybir.dt.bfloat16
_F32 = mybir.dt.float32

_program_cache: dict[tuple, tuple] = {}

# test harness hooks: extra kwargs for run_bass_kernel_spmd (e.g. trace=True)
# and the last BassKernelResults for profiling. Unused in normal grading runs.
TRACE_KWARGS: dict = {}
last_results = None

WARMUP_MM = 28


def _ru8(v: int) -> int:
    return -(-int(v) // 8) * 8


def _split_chunks(w: int) -> list[int]:
    """Split a slot width into <=512-wide chunk widths, multiples of 8."""
    if w <= 512:
        return [w]
    n = -(-w // 512)
    base = _ru8(-(-w // n))
    sizes = [base] * (n - 1)
    sizes.append(w - base * (n - 1))
    assert all(0 < s <= 512 and s % 8 == 0 for s in sizes)
    return sizes


def _pack_w1(W: np.ndarray) -> np.ndarray:
    """[D, FF] -> [128, FG*DT*FGW] with cols (g, d, j)."""
    return np.ascontiguousarray(
        W.reshape(DT, P, FG, FGW).transpose(1, 2, 0, 3).reshape(P, FG * DT * FGW)
    )


def _pack_w2(W: np.ndarray) -> np.ndarray:
    """[FF, D] -> [128, FB*FBW*D] with cols (fb, ff, j)."""
    return np.ascontiguousarray(
        W.reshape(FB, FBW, P, D).transpose(2, 0, 1, 3).reshape(P, FB * FBW * D)
    )


def _pack_x(xcols: np.ndarray, widths: list[int]) -> np.ndarray:
    """[NT, D] bf16 -> [128, DT*NT] with per-chunk col blocks (d, t)."""
    blocks = []
    off = 0
    for N in widths:
        blk = xcols[off:off + N].reshape(N, DT, P).transpose(2, 1, 0)
        blocks.append(blk.reshape(P, DT * N))
        off += N
    return np.ascontiguousarray(np.concatenate(blocks, axis=1))


def _build_program(wA: int, wB: int):
    """One SPMD Bass program: shared FFN + B-slot FFN + A-slot FFN."""
    NT = SHARED + wB + wA
    nc = bacc.Bacc()

    xp = nc.dram_tensor("xp", [P, DT * NT], _BF16, kind="ExternalInput")
    w1a = nc.dram_tensor("w1a", [P, FG * DT * FGW], _BF16, kind="ExternalInput")
    w2a = nc.dram_tensor("w2a", [P, FB * FBW * D], _BF16, kind="ExternalInput")
    w1b = nc.dram_tensor("w1b", [P, FG * DT * FGW], _BF16, kind="ExternalInput")
    w2b = nc.dram_tensor("w2b", [P, FB * FBW * D], _BF16, kind="ExternalInput")
    ws1 = nc.dram_tensor("ws1", [P, FG * DT * FGW], _BF16, kind="ExternalInput")
    ws2 = nc.dram_tensor("ws2", [P, FB * FBW * D], _BF16, kind="ExternalInput")
    b1a = nc.dram_tensor("b1a", [P, FT], _F32, kind="ExternalInput")
    b1b = nc.dram_tensor("b1b", [P, FT], _F32, kind="ExternalInput")
    bs1r = nc.dram_tensor("bs1r", [P, FT], _F32, kind="ExternalInput")
    yt = nc.dram_tensor("yt", [D, NT], _F32, kind="ExternalOutput")

    # chunks: (set_key, col_offset, width); shared first, then B, then A
    # (each later set's weights stream in during the previous phases).
    chunks = [("s", 0, SHARED)]
    off = SHARED
    for w in _split_chunks(wB):
        chunks.append(("b", off, w))
        off += w
    for w in _split_chunks(wA):
        chunks.append(("a", off, w))
        off += w
    # packed-x col offset of each chunk
    xoffs = {}
    o = 0
    for _, coff, N in chunks:
        xoffs[coff] = o
        o += DT * N

    with TileContext(nc) as tc:
        with (
            tc.tile_pool(name="wpool", bufs=1) as wpool,
            tc.tile_pool(name="xpool", bufs=1) as xpool,
            tc.tile_pool(name="hpool", bufs=34) as hpool,
            tc.tile_pool(name="ypool", bufs=8) as ypool,
            tc.tile_pool(name="bpool", bufs=1) as bpool,
            tc.tile_pool(name="psum", bufs=4, space="PSUM") as psum,
        ):
            def load_w1(src1, pfx, groups, eng=None, halves=False):
                t1 = {}
                for g in groups:
                    t = wpool.tile([P, DT * FGW], _BF16, tag=f"w1_{g}",
                                   name=f"{pfx}w1_{g}")
                    src = src1[:, g * DT * FGW:(g + 1) * DT * FGW]
                    e = eng or nc.sync
                    if halves:
                        h = DT * FGW // 2
                        e.dma_start(t[:, :h], src[:, :h])
                        e.dma_start(t[:, h:], src[:, h:])
                    else:
                        e.dma_start(t, src)
                    t1[g] = t
                return t1

            def load_w2(src2, pfx):
                t2 = []
                for fb in range(FB):
                    t = wpool.tile([P, FBW * D], _BF16, tag=f"w2_{fb}",
                                   name=f"{pfx}w2_{fb}")
                    nc.sync.dma_start(t, src2[:, fb * FBW * D:(fb + 1) * FBW * D])
                    t2.append(t)
                return t2

            def load_x(coff, N, pfx, halves=False):
                t = xpool.tile([P, DT * 512], _BF16, tag=f"x_{(coff // 512) % 2}",
                               name=f"{pfx}x")
                t = t[:, :DT * N]
                src = xp[:, xoffs[coff]:xoffs[coff] + DT * N]
                if halves:
                    h = DT * N // 2
                    nc.sync.dma_start(t[:, :h], src[:, :h])
                    nc.sync.dma_start(t[:, h:], src[:, h:])
                else:
                    nc.sync.dma_start(t, src)
                return t

            # PE warm-up: dummy matmuls on a zeroed tile (no DMA deps) keep
            # the PE busy across the DMA start so the HAM clock-gate is
            # already 8/8 (2.4 GHz) when real matmuls issue, and bridge the
            # arrival window of the critical first transfers.
            warm = bpool.tile([P, P + 512], _BF16, tag="warm", name="warm")
            nc.any.memset(warm[:, :], 0.0)
            wps = psum.tile([P, 512], _F32, tag="py", name="pwarm")
            for _ in range(WARMUP_MM):
                nc.tensor.matmul(wps, lhsT=warm[:, :P], rhs=warm[:, P:],
                                 start=True, stop=True)

            # DMA primers: the first bytes of a ring flow ~3us after issue
            # (pipeline warm-up), so lead with the tiny bias tiles on both
            # rings to absorb that latency, then the critical prefetch:
            # chunk-0 X on the sync ring, Ws1 group 0 on the scalar ring
            # (parallel issue), both split in halves so the d=0..3 slices
            # land first, then the bulk loads.
            b1bt = bpool.tile([P, FT], _F32, tag="b1b", name="b1bt")
            nc.sync.dma_start(b1bt, b1b[:, :])
            bs1t = bpool.tile([P, FT], _F32, tag="bs1", name="bs1t")
            nc.scalar.dma_start(bs1t, bs1r[:, :])
            x0 = load_x(chunks[0][1], chunks[0][2], "c0_", halves=True)
            ws1_t = load_w1(ws1, "s_", [0], eng=nc.scalar, halves=True)
            b1at = bpool.tile([P, FT], _F32, tag="b1a", name="b1at")
            nc.scalar.dma_start(b1at, b1a[:, :])

            ws1_t.update(load_w1(ws1, "s_", range(1, FG)))
            ws2_t = load_w2(ws2, "s_")
            wb_loaded = wa_loaded = None

            for ci, (kind, coff, N) in enumerate(chunks):
                if kind == "s":
                    w1t, w2t, bt = ws1_t, ws2_t, bs1t
                elif kind == "b":
                    if wb_loaded is None:
                        wb1_t = load_w1(w1b, "b_", range(FG))
                        wb2_t = load_w2(w2b, "b_")
                        wb_loaded = True
                    w1t, w2t, bt = wb1_t, wb2_t, b1bt
                else:
                    if wa_loaded is None:
                        wa1_t = load_w1(w1a, "a_", range(FG))
                        wa2_t = load_w2(w2a, "a_")
                        wa_loaded = True
                    w1t, w2t, bt = wa1_t, wa2_t, b1at

                xc = x0 if ci == 0 else load_x(coff, N, f"c{ci}_")

                hts = []
                for f in range(FT):
                    ph = psum.tile([P, 512], _F32, tag="ph", name="ph")[:, :N]
                    g, fi = divmod(f, FT // FG)
                    for d in range(DT):
                        nc.tensor.matmul(
                            ph,
                            lhsT=w1t[g][:, d * FGW + fi * P:d * FGW + (fi + 1) * P],
                            rhs=xc[:, d * N:(d + 1) * N],
                            start=(d == 0),
                            stop=(d == DT - 1),
                        )
                    ht = hpool.tile([P, 512], _BF16, tag="h", name="h")[:, :N]
                    nc.scalar.activation(
                        ht, ph, mybir.ActivationFunctionType.Gelu,
                        bias=bt[:, f:f + 1],
                    )
                    hts.append(ht)

                for d in range(DT):
                    py = psum.tile([P, 512], _F32, tag="py", name="py")[:, :N]
                    for f in range(FT):
                        fb, ff = divmod(f, FBW)
                        nc.tensor.matmul(
                            py,
                            lhsT=w2t[fb][:, ff * D + d * P:ff * D + (d + 1) * P],
                            rhs=hts[f],
                            start=(f == 0),
                            stop=(f == FT - 1),
                        )
                    yo = ypool.tile([P, 512], _F32, tag="y", name="y")[:, :N]
                    nc.vector.tensor_copy(yo, py)
                    nc.scalar.dma_start(yt[d * P:(d + 1) * P, coff:coff + N], yo)

    nc.finalize()
    return nc


def _get_program(wA: int, wB: int):
    key = (wA, wB, WARMUP_MM)
    if key not in _program_cache:
        _program_cache[key] = _build_program(wA, wB)
    return _program_cache[key]


def _route(xf: np.ndarray, W_gate: np.ndarray):
    """Replicate the reference gate in float64 (selection margins are ~1e-5,
    far above fp32 rounding, so the top-2 sets match the fp32 reference)."""
    logits = xf.astype(np.float64) @ W_gate.astype(np.float64)
    m = logits.max(axis=-1, keepdims=True)
    p = np.exp(logits - m)
    p /= p.sum(axis=-1, keepdims=True)
    top_i = np.argsort(-p, axis=-1, kind="stable")[:, :TOPK]
    top_v = np.take_along_axis(p, top_i, axis=-1)
    top_v = top_v / top_v.sum(axis=-1, keepdims=True)
    return top_i, top_v.astype(np.float32)


def _plan_cells(counts: np.ndarray):
    """Assign experts to 8 A-cells (width wA) and 8 B-cells (width wB).

    Returns (wA, wB, cells) where cells[core] = (cellA, cellB) and each
    cell = (expert, tok_start, tok_count) within the expert's token list.
    """
    order = np.argsort(-counts, kind="stable")
    big2, mid4, small2 = order[:2], order[2:6], order[6:]
    wA = _ru8(-(-int(counts[big2[0]]) // 2))
    wB = _ru8(-(-int(counts[small2[0]]) // 2))
    need = int(counts[mid4].max()) if len(mid4) else 0
    if wA + wB < need:
        wB = _ru8(need - wA)

    cellsA, cellsB = [], []
    for e in big2:
        cnt = int(counts[e])
        h = cnt // 2
        cellsA.append((int(e), 0, h))
        cellsA.append((int(e), h, cnt - h))
    for e in small2:
        cnt = int(counts[e])
        h = cnt // 2
        cellsB.append((int(e), 0, h))
        cellsB.append((int(e), h, cnt - h))
    for e in mid4:
        cnt = int(counts[e])
        s = min(wA, cnt)
        cellsA.append((int(e), 0, s))
        cellsB.append((int(e), s, cnt - s))
    assert len(cellsA) == 8 and len(cellsB) == 8
    for e, st, cn in cellsA:
        assert cn <= wA
    for e, st, cn in cellsB:
        assert cn <= wB
    return wA, wB, list(zip(cellsA, cellsB))


def kernel(x, W_gate, W1, b1, W2, b2, Ws1, bs1, Ws2, bs2):
    x = np.asarray(x, np.float32)
    xf = x.reshape(T, D)
    top_i, top_v = _route(xf, np.asarray(W_gate, np.float32))

    # per-expert token lists
    idx = [np.nonzero((top_i == e).any(axis=1))[0] for e in range(E)]
    wgt = []
    for e in range(E):
        sel = top_i[idx[e]] == e  # [cnt, K] exactly one True per row
        wgt.append(top_v[idx[e]][sel].astype(np.float32))
    counts = np.array([len(i) for i in idx])

    wA, wB, cells = _plan_cells(counts)
    NT = SHARED + wB + wA
    widths = [SHARED] + _split_chunks(wB) + _split_chunks(wA)

    xbf = xf.astype(ml_dtypes.bfloat16)
    W1 = np.asarray(W1); W2 = np.asarray(W2)
    b1 = np.asarray(b1, np.float32)
    ws1_b = _pack_w1(np.asarray(Ws1, np.float32).astype(ml_dtypes.bfloat16))
    ws2_b = _pack_w2(np.asarray(Ws2, np.float32).astype(ml_dtypes.bfloat16))
    bs1r = np.ascontiguousarray(
        np.asarray(bs1, np.float32).reshape(FT, P).T)
    w1_b = [_pack_w1(W1[e].astype(np.float32).astype(ml_dtypes.bfloat16))
            for e in range(E)]
    w2_b = [_pack_w2(W2[e].astype(np.float32).astype(ml_dtypes.bfloat16))
            for e in range(E)]
    b1_r = [np.ascontiguousarray(b1[e].reshape(FT, P).T) for e in range(E)]

    in_maps = []
    for c in range(NCORES):
        (eA, sA, nA), (eB, sB, nB) = cells[c]
        padA = np.zeros(wA, np.int64)
        padA[:nA] = idx[eA][sA:sA + nA]
        padB = np.zeros(wB, np.int64)
        padB[:nB] = idx[eB][sB:sB + nB]
        xcols = np.concatenate(
            [xbf[c * SHARED:(c + 1) * SHARED], xbf[padB], xbf[padA]], axis=0)
        in_maps.append({
            "xp": _pack_x(xcols, widths),
            "w1a": w1_b[eA], "w2a": w2_b[eA], "b1a": b1_r[eA],
            "w1b": w1_b[eB], "w2b": w2_b[eB], "b1b": b1_r[eB],
            "ws1": ws1_b, "ws2": ws2_b, "bs1r": bs1r,
        })

    nc = _get_program(wA, wB)
    global last_results
    last_results = run_bass_kernel_spmd(
        nc, in_maps, list(range(NCORES)), **TRACE_KWARGS)
    res = last_results.results

    out = np.zeros((T, D), np.float32)
    for c in range(NCORES):
        (eA, sA, nA), (eB, sB, nB) = cells[c]
        y = np.asarray(res[c]["yt"], np.float32)
        out[c * SHARED:(c + 1) * SHARED] += y[:, :SHARED].T
        tb = idx[eB][sB:sB + nB]
        out[tb] += wgt[eB][sB:sB + nB, None] * y[:, SHARED:SHARED + nB].T
        ta = idx[eA][sA:sA + nA]
        out[ta] += wgt[eA][sA:sA + nA, None] * \
            y[:, SHARED + wB:SHARED + wB + nA].T

    # biases enter linearly; add on host (zeros in this problem's inputs)
    b2 = np.asarray(b2, np.float32)
    bs2 = np.asarray(bs2, np.float32)
    combine = np.zeros((T, E), np.float32)
    np.put_along_axis(combine, top_i, top_v, axis=1)
    out += combine @ b2 + bs2

    return out.reshape(B, L, D)
